# revision 48
# baseline (speedup 1.0000x reference)
"""Trainium2 Bass kernel for AnyGPT local-attention block (8 NeuronCores).

Sharding: (batch, seq-half) -> 8 shards of 1024 query tokens each; every core
gets a 256-token k/v halo (zero-padded at sequence start), so no collectives
are needed and the host gather is a pure concatenation.

Per-core pipeline (all matmuls in bf16, LayerNorm/softmax math in fp32):
  qT/kT = W^T-major projections ([H, tok] layout), v natural ([tok, H]) with a
  built-in ones column for softmax denominators; banded scores computed
  TRANSPOSED ([key, query] layout) so exp'd probs feed the ctx matmul directly
  and ctx comes out pre-transposed for the output projection; softmax is
  unnormalized (no max subtraction; scores are O(30)) with the denominator
  recovered from the ones row and divided into ctx via a rank-1 broadcast.
"""

import numpy as np
import ml_dtypes

import concourse.bass as bass
import concourse.mybir as mybir
import concourse.tile as tile
from concourse import bacc

F32 = mybir.dt.float32
BF16 = mybir.dt.bfloat16

B, S, H, NH, HD, WIN = 4, 2048, 1024, 16, 64, 256
P = 128
SQ = 1024          # queries per core
SE = SQ + WIN      # ext tokens (halo + queries)
KB = H // P        # 8 contraction blocks
QBS = 256          # query block size in attention
NQB = SQ // QBS    # 4
JBN = 4            # key blocks of 128 per query block
LN_EPS = 1e-7
NCORES = 8

AF = mybir.ActivationFunctionType
ALU = mybir.AluOpType


def _bcast_ap(handle, n_part):
    """[D] DRAM vector -> [n_part, D] partition-broadcast AP (step 0)."""
    ap = handle[:]
    return bass.AP(tensor=ap.tensor, offset=ap.offset, ap=[[0, n_part]] + list(ap.ap))


def build_nc():
    nc = bacc.Bacc("TRN2", target_bir_lowering=False, debug=False)

    xq_h = nc.declare_dram_parameter("xq", [SQ, H], F32, isOutput=False)
    xT_h = nc.declare_dram_parameter("xT", [H, SE], BF16, isOutput=False)
    wqT_h = nc.declare_dram_parameter("wqT", [H, H], BF16, isOutput=False)
    wkT_h = nc.declare_dram_parameter("wkT", [H, H], BF16, isOutput=False)
    wvT_h = nc.declare_dram_parameter("wvT", [H, H], BF16, isOutput=False)
    woT_h = nc.declare_dram_parameter("woT", [H, H], BF16, isOutput=False)
    lnw_h = nc.declare_dram_parameter("lnw", [H], F32, isOutput=False)
    lnbbo_h = nc.declare_dram_parameter("lnbbo", [H], F32, isOutput=False)
    vones_h = nc.declare_dram_parameter("vones", [SE], BF16, isOutput=False)
    sel_h = nc.declare_dram_parameter("sel", [NH, KB, P], BF16, isOutput=False)
    out_h = nc.declare_dram_parameter("out", [SQ, H], F32, isOutput=True)

    with tile.TileContext(nc) as tc:
        _body(tc, nc, xq_h, xT_h, wqT_h, wkT_h, wvT_h, woT_h, lnw_h, lnbbo_h,
              vones_h, sel_h, out_h)
    nc.compile()
    return nc


def _body(tc, nc, xq_h, xT_h, wqT_h, wkT_h, wvT_h, woT_h, lnw_h, lnbbo_h,
          vones_h, sel_h, out_h):
    with (
        tc.tile_pool(name="const", bufs=1) as const,
        tc.tile_pool(name="big", bufs=1) as big,
        tc.tile_pool(name="wpool", bufs=16) as wpool,
        tc.tile_pool(name="work", bufs=3) as work,
        tc.tile_pool(name="lnpool", bufs=2) as lnpool,
    ):
        # residual parks in DRAM between LN (early) and the final add (late);
        # SBUF is too tight to hold 4 MB of fp32 for the whole kernel
        res_dram = nc.dram_tensor("res_dram", [SQ, H], F32)
        # ---- constants ----
        lnw_b = const.tile([P, H], F32)
        nc.sync.dma_start(lnw_b[:], _bcast_ap(lnw_h, P))
        lnbbo_b = const.tile([P, H], F32)
        nc.sync.dma_start(lnbbo_b[:], _bcast_ap(lnbbo_h, P))
        eps_t = const.tile([P, 1], F32)
        nc.vector.memset(eps_t[:], LN_EPS)
        sel_sb = const.tile([NH, KB, P], BF16)
        nc.sync.dma_start(sel_sb[:], sel_h[:][:, :, :])
        # jb0 band mask (keep where c < r) as a tile so DVE can apply it,
        # taking one of the four per-pair affine_selects off the Pool engine
        mask0 = const.tile([P, QBS], BF16)
        nc.vector.memset(mask0[:], 1.0)
        nc.gpsimd.affine_select(out=mask0[:], in_=mask0[:],
                                compare_op=ALU.is_ge, fill=0.0, base=-1,
                                pattern=[[-1, QBS]], channel_multiplier=1)

        # ---- x^T resident [128, kb, tok]; interleave with the first weight
        # loads and split halves so the first matmul's operands land early ----
        xt_sb = big.tile([P, KB, SE], BF16)
        wq_sl = [wpool.tile([P, H], BF16, tag="wslice", name=f"wq_{kb}")
                 for kb in range(KB)]
        for kb in range(KB):
            nc.sync.dma_start(wq_sl[kb][:], wqT_h[:][kb * P:(kb + 1) * P, :])
            half = SE // 2
            nc.sync.dma_start(xt_sb[:, kb, :half],
                              xT_h[:][kb * P:(kb + 1) * P, :half])
            nc.sync.dma_start(xt_sb[:, kb, half:],
                              xT_h[:][kb * P:(kb + 1) * P, half:])

        qT_sb = big.tile([P, KB, SQ], BF16)    # q^T  [H, 1024]
        kT_sb = big.tile([P, KB, SE], BF16)    # k^T  [H, 1280]
        # v natural + a "ones" column that is 0.0 for zero-padded halo tokens,
        # so pad keys contribute exactly nothing to ctx or the denominators
        v_sb = big.tile([P, SE // P, NH, HD + 1], BF16)
        vo = vones_h[:]
        vo_pt = bass.AP(tensor=vo.tensor, offset=vo.offset,
                        ap=[[1, P], [P, SE // P]])
        for h in range(NH):
            nc.sync.dma_start(v_sb[:, :, h, HD], vo_pt)
        ct_sb = big.tile([P, KB, SQ], BF16)    # UNnormalized ctx^T [H, 1024]
        den_sb = big.tile([NH, SQ], F32)       # softmax denominators [head, i]
        recip_sb = big.tile([NH, SQ], BF16)    # 1/den, bulk-reciprocated

        with tc.tile_pool(name="ppsum", bufs=4, space="PSUM") as ppsum:
            # ---- transposed projections: q^T, k^T ----
            for (w_h, dst, tok0, ntok, wsl) in ((wqT_h, qT_sb, WIN, SQ, wq_sl),
                                                (wkT_h, kT_sb, 0, SE, None)):
                if wsl is None:
                    wsl = [wpool.tile([P, H], BF16, tag="wslice",
                                      name=f"w_{kb}") for kb in range(KB)]
                    for kb in range(KB):
                        nc.sync.dma_start(wsl[kb][:],
                                          w_h[:][kb * P:(kb + 1) * P, :])
                chunks = [(i, min(512, ntok - i)) for i in range(0, ntok, 512)]
                for ob in range(KB):
                    for (i0, ilen) in chunks:
                        ps = ppsum.tile([P, 512], F32, tag="pj", name="ps_qk")
                        for kb in range(KB):
                            nc.tensor.matmul(
                                ps[:, :ilen],
                                wsl[kb][:, ob * P:(ob + 1) * P],
                                xt_sb[:, kb, tok0 + i0: tok0 + i0 + ilen],
                                start=(kb == 0), stop=(kb == KB - 1),
                            )
                        nc.scalar.copy(out=dst[:, ob, i0:i0 + ilen],
                                       in_=ps[:, :ilen])

            # ---- natural projection: v ----
            wsl = [wpool.tile([P, H], BF16, tag="wslice", name=f"wv_{kb}")
                   for kb in range(KB)]
            for kb in range(KB):
                nc.sync.dma_start(wsl[kb][:], wvT_h[:][kb * P:(kb + 1) * P, :])
            for tt in range(SE // P):
                for oh in range(2):
                    ps = ppsum.tile([P, 512], F32, tag="pj", name="ps_v")
                    for kb in range(KB):
                        nc.tensor.matmul(
                            ps[:],
                            xt_sb[:, kb, tt * P:(tt + 1) * P],
                            wsl[kb][:, oh * 512:(oh + 1) * 512],
                            start=(kb == 0), stop=(kb == KB - 1),
                        )
                    nc.scalar.copy(
                        out=v_sb[:, tt, oh * 8:(oh + 1) * 8, 0:HD],
                        in_=ps[:].rearrange("p (h d) -> p h d", d=HD),
                    )

            # ---- wo slices (prefetch; consumed at the end) ----
            wosl = [wpool.tile([P, H], BF16, tag="wslice", name=f"wo_{kb}")
                    for kb in range(KB)]
            for kb in range(KB):
                nc.sync.dma_start(wosl[kb][:], woT_h[:][kb * P:(kb + 1) * P, :])

            # ---- LayerNorm residual (DVE is idle during projections) ----
            for it in range(KB):
                x_t = lnpool.tile([P, H], F32, tag="x_t", name="x_t")
                nc.sync.dma_start(x_t[:], xq_h[:][it * P:(it + 1) * P, :])
                stats = lnpool.tile([P, 2, 6], F32, tag="stats", name="stats")
                for g in range(2):
                    nc.vector.bn_stats(out=stats[:, g, :],
                                       in_=x_t[:, g * 512:(g + 1) * 512])
                mv = lnpool.tile([P, 2], F32, tag="mv", name="mv")
                nc.vector.bn_aggr(out=mv[:], in_=stats[:])
                std = lnpool.tile([P, 1], F32, tag="std", name="std")
                nc.scalar.activation(out=std[:], in_=mv[:, 1:2], func=AF.Sqrt,
                                     bias=eps_t[:])
                rstd = lnpool.tile([P, 1], F32, tag="rstd", name="rstd")
                nc.vector.reciprocal(out=rstd[:], in_=std[:])
                res_t = lnpool.tile([P, H], F32, tag="res_t", name="res_t")
                nc.vector.tensor_scalar(out=res_t[:], in0=x_t[:],
                                        scalar1=mv[:, 0:1], scalar2=rstd[:],
                                        op0=ALU.subtract, op1=ALU.mult)
                nc.vector.tensor_mul(out=res_t[:], in0=res_t[:], in1=lnw_b[:])
                nc.vector.tensor_add(out=res_t[:], in0=res_t[:], in1=lnbbo_b[:])
                nc.sync.dma_start(res_dram[it * P:(it + 1) * P, :], res_t[:])

        # ---- attention: scores^T -> exp -> mask -> ctx^T -> normalize ----
        # Head-PAIR iterations: the even head's score matmuls contract on PE
        # rows 0-63, the odd head's on rows 64-127 (tile_position auto-derived
        # from the lhsT base partition), writing different PSUM banks, so the
        # hardware runs each jb's pair concurrently. Software-pipelined with a
        # 2-pair lookahead so the in-order PE never waits on exp/mask.
        with (
            tc.tile_pool(name="spsum", bufs=3, space="PSUM") as spsum,
            tc.tile_pool(name="cpsum", bufs=2, space="PSUM") as cpsum,
        ):
            pairs = [(qb, hb) for qb in range(NQB) for hb in range(NH // 2)]
            probs_of = {}

            def emit_scores(i):
                qb, hb = pairs[i]
                probs = work.tile([P, 2, JBN, QBS], BF16, tag="probs",
                                  name="probs", bufs=4)
                # two half-tiles of 2 jb x 2 parities (2 PSUM banks each) so
                # exp can drain each half while the next one is computed
                for half in range(2):
                    ps_s = spsum.tile([P, 2, 2, QBS], F32, tag="sc",
                                      name="ps_s")
                    for jbh in range(2):
                        jb = 2 * half + jbh
                        j0 = qb * QBS + jb * P
                        for par in range(2):
                            ho = par * HD
                            nc.tensor.matmul(
                                ps_s[:, par, jbh, :],
                                kT_sb[ho:ho + HD, hb, j0:j0 + P],
                                qT_sb[ho:ho + HD, hb,
                                      qb * QBS:(qb + 1) * QBS],
                                start=True, stop=True,
                            )
                    nc.scalar.activation(
                        out=probs[:, :, 2 * half:2 * half + 2, :],
                        in_=ps_s[:], func=AF.Exp)
                # band mask: each jb block is a single affine inequality over
                # (key row r, query col c), applied in place on the idle Pool
                # engine. r = partition, parity is a dead dim (step 0), c is
                # the last free dim. Keep where A >= 0, zero elsewhere:
                #   jb0: c<r (DVE, via mask0)   jb1: r-c+127>=0   jb2: c-r>=0
                #   jb3: c-r-128>=0
                m0 = bass.AP(tensor=mask0.tensor, offset=mask0.offset,
                             ap=[mask0.ap[0], [0, 2], mask0.ap[1]])
                nc.vector.tensor_mul(out=probs[:, :, 0, :],
                                     in0=probs[:, :, 0, :], in1=m0)
                for jb, (ch, cstep, base) in ((1, (1, -1, 127)),
                                              (2, (-1, 1, 0)),
                                              (3, (-1, 1, -128))):
                    nc.gpsimd.affine_select(
                        out=probs[:, :, jb, :], in_=probs[:, :, jb, :],
                        compare_op=ALU.is_ge, fill=0.0, base=base,
                        pattern=[[0, 2], [cstep, QBS]],
                        channel_multiplier=ch)
                probs_of[i] = probs

            def emit_ctx(i):
                qb, hb = pairs[i]
                probs = probs_of.pop(i)
                ps_c = cpsum.tile([HD + 1, 2, QBS], F32, tag="cx", name="ps_c")
                for par in range(2):
                    for jb in range(JBN):
                        nc.tensor.matmul(
                            ps_c[:, par, :],
                            v_sb[:, qb * 2 + jb, 2 * hb + par, :],
                            probs[:, par, jb, :],
                            start=(jb == 0), stop=(jb == JBN - 1),
                        )
                qs = slice(qb * QBS, (qb + 1) * QBS)
                nc.vector.tensor_copy(out=ct_sb[0:HD, hb, qs],
                                      in_=ps_c[0:HD, 0, :])
                nc.vector.tensor_copy(out=ct_sb[HD:P, hb, qs],
                                      in_=ps_c[0:HD, 1, :])
                dstage = work.tile([1, 2, QBS], F32, tag="dstage",
                                   name="dstage")
                nc.vector.tensor_copy(out=dstage[:], in_=ps_c[HD:HD + 1, :, :])
                for par in range(2):
                    nc.sync.dma_start(
                        out=den_sb[2 * hb + par:2 * hb + par + 1, qs],
                        in_=dstage[:, par, :])

            emit_scores(0)
            emit_scores(1)
            for i in range(len(pairs)):
                if i + 2 < len(pairs):
                    emit_scores(i + 2)
                emit_ctx(i)
                if (i + 1) % (NH // 2) == 0:
                    # all heads of this query block done: reciprocate its
                    # denominator slice now so out-proj never waits on it
                    qb = pairs[i][0]
                    qs = slice(qb * QBS, (qb + 1) * QBS)
                    with nc.allow_low_precision(
                            reason="softmax denom recip in bf16: 0.4% rel "
                                   "on a 2e-2 budget"):
                        nc.vector.reciprocal(out=recip_sb[:, qs],
                                             in_=den_sb[:, qs])

        # ---- normalize ctx^T, then output projection + residual ----
        # R = selector-matmul broadcast of the per-head reciprocals into the
        # [128, 128] block layout of ct_sb (rows 0-63 <- even head, 64-127 <-
        # odd head), then ct_sb *= R in place.
        with (
            tc.tile_pool(name="opsum", bufs=4, space="PSUM") as opsum,
            tc.tile_pool(name="rpsum", bufs=4, space="PSUM") as rpsum,
        ):
            for it in range(KB):
                isl = slice(it * P, (it + 1) * P)
                for hb in range(KB):
                    ps_r = rpsum.tile([P, P], F32, tag="r", name="ps_r")
                    nc.tensor.matmul(ps_r[:], sel_sb[:, hb, :],
                                     recip_sb[:, isl], start=True, stop=True)
                    nc.vector.tensor_mul(out=ct_sb[:, hb, isl],
                                         in0=ct_sb[:, hb, isl], in1=ps_r[:])
            for it in range(KB):
                for oh in range(2):
                    ps_o = opsum.tile([P, 512], F32, tag="po", name="ps_o")
                    for hb in range(KB):
                        nc.tensor.matmul(
                            ps_o[:],
                            ct_sb[:, hb, it * P:(it + 1) * P],
                            wosl[hb][:, oh * 512:(oh + 1) * 512],
                            start=(hb == 0), stop=(hb == KB - 1),
                        )
                    o_t = work.tile([P, 512], F32, tag="o_t", name="o_t")
                    nc.scalar.copy(out=o_t[:], in_=ps_o[:])
                    osl = out_h[:][it * P:(it + 1) * P,
                                   oh * 512:(oh + 1) * 512]
                    nc.sync.dma_start(osl, o_t[:])
                    # residual folded in by the DMA's inline adder
                    nc.gpsimd.dma_start(
                        osl,
                        res_dram[it * P:(it + 1) * P, oh * 512:(oh + 1) * 512],
                        accum_op=ALU.add)


_CACHE = {}


def get_nc():
    if "nc" not in _CACHE:
        _CACHE["nc"] = build_nc()
    return _CACHE["nc"]


def make_in_maps(inputs):
    x = np.asarray(inputs["hidden_states"], dtype=np.float32)
    wq = np.asarray(inputs["wq"], dtype=np.float32)
    wk = np.asarray(inputs["wk"], dtype=np.float32)
    wv = np.asarray(inputs["wv"], dtype=np.float32)
    wo = np.asarray(inputs["wo"], dtype=np.float32)
    bo = np.asarray(inputs["bo"], dtype=np.float32)
    ln_w = np.asarray(inputs["ln_w"], dtype=np.float32)
    ln_b = np.asarray(inputs["ln_b"], dtype=np.float32)

    bf = ml_dtypes.bfloat16
    wqT = np.ascontiguousarray(wq.T).astype(bf)
    wkT = np.ascontiguousarray(wk.T).astype(bf)
    wvT = np.ascontiguousarray(wv.T).astype(bf)
    woT = np.ascontiguousarray(wo.T).astype(bf)
    lnbbo = (ln_b + bo).astype(np.float32)

    # selector for the reciprocal broadcast: sel[p, hb, m] = 1 iff head p owns
    # row m of h-block hb in the ct layout (even head -> rows 0-63, odd -> 64+)
    sel = np.zeros((NH, KB, P), dtype=np.float32)
    for hb in range(KB):
        sel[2 * hb, hb, :HD] = 1.0
        sel[2 * hb + 1, hb, HD:] = 1.0
    sel = sel.astype(bf)

    in_maps = []
    for core in range(NCORES):
        b, hh = divmod(core, 2)
        start = hh * SQ
        xkv = np.zeros((SE, H), dtype=np.float32)
        xkv[WIN:] = x[b, start:start + SQ]
        vones = np.ones(SE, dtype=np.float32)
        if start > 0:
            xkv[:WIN] = x[b, start - WIN:start]
        else:
            vones[:WIN] = 0.0
        in_maps.append({
            "xq": np.ascontiguousarray(x[b, start:start + SQ]),
            "xT": np.ascontiguousarray(xkv.T).astype(bf),
            "wqT": wqT, "wkT": wkT, "wvT": wvT, "woT": woT,
            "lnw": ln_w, "lnbbo": lnbbo,
            "vones": vones.astype(bf),
            "sel": sel,
        })
    return in_maps


def kernel(**inputs):
    from concourse.bass_utils import run_bass_kernel_spmd
    nc = get_nc()
    in_maps = make_in_maps(inputs)
    res = run_bass_kernel_spmd(nc, in_maps, core_ids=list(range(NCORES)))
    out = np.empty((B, S, H), dtype=np.float32)
    for core in range(NCORES):
        b, hh = divmod(core, 2)
        out[b, hh * SQ:(hh + 1) * SQ, :] = res.results[core]["out"]
    return out
